# revision 7
# baseline (speedup 1.0000x reference)
"""Trainium2 Bass kernel for CrossSparseGAT message passing (8 NeuronCores).

Strategy (edge-parallel, dst-sorted):
  - Host: sort edges by dst, partition dst range across 8 cores (6250 dsts
    each), group each core's edges into 49 blocks of 128 dsts, pad each
    block's edge list to C chunks of 128 edges.
  - Device, per core:
      Phase A : project this core's src slice:  VA = src_feats @ [Wv | W2@W4]
      AllGather VA shards -> full VA table [50000, 136] on every core.
      Phase A2: a_dst = dst_feats_slice @ (W1@W4)  -> DRAM table [6250, 8]
      Phase C : per block: indirect-gather VA rows by src, CCE-add-gather
                a_dst rows by dst, compute per-edge softmax weights
                w = exp(leakyrelu(z)), scatter via one-hot matmul
                (S^T @ [w*V | w]) accumulated in PSUM over C chunks,
                normalize by the per-dst weight sum -> agg kept in SBUF.
      Phase D : out = agg @ Wout_w + dst_feats @ res_w + bias, LayerNorm.
  - Host: concatenate the 8 per-core output slices.

The segment softmax is computed without max-subtraction: exp(l - m) /
sum(exp(l - m)) == exp(l)/sum(exp(l)) and logits are O(10) here, so fp32
exp is safe (validated against an fp64 reference: rel err ~2e-7).
"""

import os

import numpy as np

N_DST = 50000
N_SRC = 50000
E = 500000
D = 128
NH = 8
HD = D // NH
NCORES = 8
PER = N_DST // NCORES          # 6250 dsts per core
NBLK = (PER + 127) // 128      # 49 blocks of 128 dsts
LAST_ROWS = PER - 128 * (NBLK - 1)  # rows in the last (partial) block
F_VA = D + NH                  # 136: [V | a_src] row size

# results of the last kernel() call, for the test harness
LAST_RUN = {}


def _prep_edges(edge_index, P_edge, deter_edge, w34):
    """Sort edges by dst, shard by dst range, pad per (core, block).

    Returns (C, ezb[8, NBLK, 128, C, 9] f32, eidx[8, NBLK, 128, C, 2] i32).
    ezb[..., 0:8] = P*w34 + deter (per-edge logit bias), ezb[..., 8] = dst
    offset within the block (999 marks padding -> one-hot row is all zero).
    eidx[..., 0] = global src index, eidx[..., 1] = dst index within core.
    """
    src = np.asarray(edge_index[0], dtype=np.int64)
    dst = np.asarray(edge_index[1], dtype=np.int64)
    order = np.argsort(dst, kind="stable")
    ssrc = src[order]
    sdst = dst[order]
    zb = (np.asarray(P_edge, np.float32)[order, None] * w34[None, :]
          + np.asarray(deter_edge, np.float32)[order, None]).astype(np.float32)

    core = sdst // PER
    local = sdst - core * PER
    blk = local // 128
    rel = local - blk * 128
    flat = core * NBLK + blk
    counts = np.bincount(flat, minlength=NCORES * NBLK)
    C = int(np.ceil(counts.max() / 128))
    C = max(C, 2)
    cap = C * 128

    start = np.zeros(NCORES * NBLK, np.int64)
    np.cumsum(counts[:-1], out=start[1:])
    slot = np.arange(E, dtype=np.int64) - start[flat]

    ezb = np.zeros((NCORES, NBLK, cap, 9), np.float32)
    ezb[:, :, :, 8] = 999.0
    eidx = np.zeros((NCORES, NBLK, cap, 2), np.int32)
    ezb[core, blk, slot, 0:8] = zb
    ezb[core, blk, slot, 8] = rel.astype(np.float32)
    eidx[core, blk, slot, 0] = ssrc.astype(np.int32)
    eidx[core, blk, slot, 1] = local.astype(np.int32)

    # device layout: [core, blk, partition(=slot%128), chunk(=slot//128), f]
    ezb = np.ascontiguousarray(
        ezb.reshape(NCORES, NBLK, C, 128, 9).transpose(0, 1, 3, 2, 4))
    eidx = np.ascontiguousarray(
        eidx.reshape(NCORES, NBLK, C, 128, 2).transpose(0, 1, 3, 2, 4))
    return C, ezb, eidx


def _build_program(C):
    import concourse.bass as bass
    import concourse.bacc as bacc
    import concourse.tile as tile
    from concourse import mybir
    from concourse.masks import make_identity

    f32 = mybir.dt.float32
    i32 = mybir.dt.int32
    A = mybir.AluOpType

    nc = bacc.Bacc(num_devices=NCORES)

    # --- I/O ---
    srcf = nc.dram_tensor("srcf", [PER, D], f32, kind="ExternalInput")
    dstf = nc.dram_tensor("dstf", [PER, D], f32, kind="ExternalInput")
    ezb = nc.dram_tensor("ezb", [NBLK, 128, C, 9], f32, kind="ExternalInput")
    eidx = nc.dram_tensor("eidx", [NBLK, 128, C, 2], i32, kind="ExternalInput")
    wva = nc.dram_tensor("wva", [D, F_VA], f32, kind="ExternalInput")
    w14 = nc.dram_tensor("w14", [D, NH], f32, kind="ExternalInput")
    woutw = nc.dram_tensor("woutw", [D, D], f32, kind="ExternalInput")
    resw = nc.dram_tensor("resw", [D, D], f32, kind="ExternalInput")
    biasv = nc.dram_tensor("biasv", [D], f32, kind="ExternalInput")
    lngv = nc.dram_tensor("lngv", [D], f32, kind="ExternalInput")
    lnbv = nc.dram_tensor("lnbv", [D], f32, kind="ExternalInput")
    y = nc.dram_tensor("y", [PER, D], f32, kind="ExternalOutput")

    def row_bcast(h):
        # DRAM [D] -> broadcast across 128 partitions
        ap = h[:]
        return bass.AP(tensor=ap.tensor, offset=ap.offset,
                       ap=[[0, 128]] + list(ap.ap))

    with tile.TileContext(nc) as tc:
        with (
            tc.tile_pool(name="consts", bufs=1) as consts,
            tc.tile_pool(name="aggp", bufs=1) as aggp,
            tc.tile_pool(name="densew", bufs=2) as densew,
            tc.tile_pool(name="edgew", bufs=3) as edgew,
            tc.tile_pool(name="psT", bufs=2, space="PSUM") as psT,
            tc.tile_pool(name="psMM", bufs=2, space="PSUM") as psMM,
            tc.tile_pool(name="psC", bufs=2, space="PSUM") as psC,
            tc.tile_pool(name="dram", bufs=1, space="DRAM") as dram,
        ):
            # --- constants ---
            ident = consts.tile([128, 128], f32)
            make_identity(nc, ident[:])
            iota_i = consts.tile([128, 128], i32)
            nc.gpsimd.iota(iota_i[:], pattern=[[1, 128]], base=0,
                           channel_multiplier=0)
            iota_f = consts.tile([128, 128], f32)
            nc.vector.tensor_copy(iota_f[:], iota_i[:])
            wva_sb = consts.tile([128, F_VA], f32)
            nc.sync.dma_start(out=wva_sb[:], in_=wva[:, :])
            w14_sb = consts.tile([128, NH], f32)
            nc.sync.dma_start(out=w14_sb[:], in_=w14[:, :])
            woutw_sb = consts.tile([128, D], f32)
            nc.sync.dma_start(out=woutw_sb[:], in_=woutw[:, :])
            resw_sb = consts.tile([128, D], f32)
            nc.sync.dma_start(out=resw_sb[:], in_=resw[:, :])
            bias_row = consts.tile([128, D], f32)
            nc.sync.dma_start(out=bias_row[:], in_=row_bcast(biasv))
            lng_row = consts.tile([128, D], f32)
            nc.sync.dma_start(out=lng_row[:], in_=row_bcast(lngv))
            lnb_row = consts.tile([128, D], f32)
            nc.sync.dma_start(out=lnb_row[:], in_=row_bcast(lnbv))
            eps12 = consts.tile([128, 1], f32)
            nc.vector.memset(eps12[:], 1e-12)
            epsln = consts.tile([128, 1], f32)
            nc.vector.memset(epsln[:], 1e-5)

            # SBUF-resident per-core aggregate [dst_in_block(part), blk*feat]
            aggbig = aggp.tile([128, NBLK * D], f32)

            # DRAM scratch
            va_sh = dram.tile([PER, F_VA], f32)
            va_full = dram.tile([N_SRC, F_VA], f32, addr_space="Shared")
            adst_loc = dram.tile([PER, NH], f32)

            # --- Phase A: VA shard = src_slice @ [Wv | W24] ---
            for t in range(NBLK):
                r0 = t * 128
                r1 = min(r0 + 128, PER)
                n = r1 - r0
                ft = densew.tile([128, D], f32, tag="ft")
                nc.sync.dma_start(out=ft[:n, :], in_=srcf[r0:r1, :])
                ftT_p = psT.tile([128, 128], f32, tag="tp")
                nc.tensor.transpose(ftT_p[:], ft[:], ident[:])
                ftT = densew.tile([128, 128], f32, tag="ftT")
                nc.vector.tensor_copy(ftT[:], ftT_p[:])
                va_p = psMM.tile([128, F_VA], f32, tag="mm")
                nc.tensor.matmul(va_p[:], lhsT=ftT[:], rhs=wva_sb[:],
                                 start=True, stop=True)
                va_sb = densew.tile([128, F_VA], f32, tag="vasb")
                nc.vector.tensor_copy(va_sb[:], va_p[:])
                nc.sync.dma_start(out=va_sh[r0:r1, :], in_=va_sb[:n, :])

            # --- AllGather the VA table ---
            nc.gpsimd.collective_compute(
                "AllGather",
                mybir.AluOpType.bypass,
                replica_groups=[list(range(NCORES))],
                ins=[va_sh[:].opt()],
                outs=[va_full[:].opt()],
            )

            # --- Phase A2: a_dst table for this core's dst slice ---
            for t in range(NBLK):
                r0 = t * 128
                r1 = min(r0 + 128, PER)
                n = r1 - r0
                dt_ = densew.tile([128, D], f32, tag="ft")
                nc.sync.dma_start(out=dt_[:n, :], in_=dstf[r0:r1, :])
                dtT_p = psT.tile([128, 128], f32, tag="tp")
                nc.tensor.transpose(dtT_p[:], dt_[:], ident[:])
                dtT = densew.tile([128, 128], f32, tag="ftT")
                nc.vector.tensor_copy(dtT[:], dtT_p[:])
                ad_p = psMM.tile([128, NH], f32, tag="mm")
                nc.tensor.matmul(ad_p[:], lhsT=dtT[:], rhs=w14_sb[:],
                                 start=True, stop=True)
                ad_sb = densew.tile([128, NH], f32, tag="adsb")
                nc.vector.tensor_copy(ad_sb[:], ad_p[:])
                nc.sync.dma_start(out=adst_loc[r0:r1, :], in_=ad_sb[:n, :])

            # --- Phase C: edge processing, one block of 128 dsts at a time ---
            for b in range(NBLK):
                ez = edgew.tile([128, C, 9], f32, tag="ez")
                nc.sync.dma_start(out=ez[:], in_=ezb[b])
                ei = edgew.tile([128, C, 2], i32, tag="ei")
                nc.sync.dma_start(out=ei[:], in_=eidx[b])

                vab = edgew.tile([128, C, F_VA], f32, tag="vab")
                for k in range(C):
                    nc.gpsimd.indirect_dma_start(
                        out=vab[:, k, :],
                        out_offset=None,
                        in_=va_full[:],
                        in_offset=bass.IndirectOffsetOnAxis(
                            ap=ei[:, k, 0:1], axis=0),
                    )
                for k in range(C):
                    nc.gpsimd.indirect_dma_start(
                        out=vab[:, k, D:F_VA],
                        out_offset=None,
                        in_=adst_loc[:],
                        in_offset=bass.IndirectOffsetOnAxis(
                            ap=ei[:, k, 1:2], axis=0),
                        compute_op=A.add,
                    )

                # z = (P*w34 + deter) + (a_src + a_dst);  l = max(z, 0.2 z)
                zt = edgew.tile([128, C, NH], f32, tag="zt")
                nc.vector.tensor_tensor(zt[:], ez[:, :, 0:8],
                                        vab[:, :, D:F_VA], A.add)
                lt = edgew.tile([128, C, NH], f32, tag="lt")
                nc.vector.scalar_tensor_tensor(lt[:], zt[:], 0.2, zt[:],
                                               A.mult, A.max)
                pay = edgew.tile([128, C, F_VA], f32, tag="pay")
                nc.scalar.activation(pay[:, :, D:F_VA], lt[:],
                                     mybir.ActivationFunctionType.Exp)
                # msgs = w (per head) * V
                nc.vector.tensor_tensor(
                    pay[:, :, 0:D].rearrange("p c (h j) -> p c h j", h=NH),
                    vab[:, :, 0:D].rearrange("p c (h j) -> p c h j", h=NH),
                    pay[:, :, D:F_VA].unsqueeze(3).to_broadcast(
                        [128, C, NH, HD]),
                    A.mult)
                # one-hot S[e, d] = (dst_rel[e] == d)
                St = edgew.tile([128, C, 128], f32, tag="St")
                nc.vector.tensor_tensor(
                    St[:],
                    ez[:, :, 8:9].to_broadcast([128, C, 128]),
                    iota_f[:].unsqueeze(1).to_broadcast([128, C, 128]),
                    A.is_equal)

                ps = psC.tile([128, F_VA], f32, tag="ps")
                for k in range(C):
                    nc.tensor.matmul(ps[:], lhsT=St[:, k, :], rhs=pay[:, k, :],
                                     start=(k == 0), stop=(k == C - 1))

                # normalize: agg = U / (ssum + 1e-12)
                rec = edgew.tile([128, NH], f32, tag="rec")
                nc.scalar.activation(rec[:], ps[:, D:F_VA],
                                     mybir.ActivationFunctionType.Identity,
                                     bias=eps12[:])
                nc.vector.reciprocal(rec[:], rec[:])
                nc.vector.tensor_tensor(
                    aggbig[:, b * D:(b + 1) * D].rearrange(
                        "p (h j) -> p h j", h=NH),
                    ps[:, 0:D].rearrange("p (h j) -> p h j", h=NH),
                    rec[:].unsqueeze(2).to_broadcast([128, NH, HD]),
                    A.mult)

            # --- Phase D: out = agg @ Wout_w + dstf @ res_w + bias; LayerNorm
            for t in range(NBLK):
                r0 = t * 128
                r1 = min(r0 + 128, PER)
                n = r1 - r0
                agT_p = psT.tile([128, 128], f32, tag="tp")
                nc.tensor.transpose(agT_p[:], aggbig[:, t * D:(t + 1) * D],
                                    ident[:])
                agT = densew.tile([128, 128], f32, tag="ftT")
                nc.vector.tensor_copy(agT[:], agT_p[:])
                dt_ = densew.tile([128, D], f32, tag="ft")
                nc.sync.dma_start(out=dt_[:n, :], in_=dstf[r0:r1, :])
                dtT_p = psT.tile([128, 128], f32, tag="tp")
                nc.tensor.transpose(dtT_p[:], dt_[:], ident[:])
                dtT = densew.tile([128, 128], f32, tag="ftT2")
                nc.vector.tensor_copy(dtT[:], dtT_p[:])
                op = psMM.tile([128, D], f32, tag="mm")
                nc.tensor.matmul(op[:], lhsT=agT[:], rhs=woutw_sb[:],
                                 start=True, stop=False)
                nc.tensor.matmul(op[:], lhsT=dtT[:], rhs=resw_sb[:],
                                 start=False, stop=True)
                xt = densew.tile([128, D], f32, tag="xt")
                nc.vector.tensor_tensor(xt[:], op[:], bias_row[:], A.add)
                stats = densew.tile([128, nc.vector.BN_STATS_DIM], f32,
                                    tag="stats")
                nc.vector.bn_stats(stats[:], xt[:])
                mv = densew.tile([128, nc.vector.BN_AGGR_DIM], f32, tag="mv")
                nc.vector.bn_aggr(mv[:], stats[:])
                rstd = densew.tile([128, 1], f32, tag="rstd")
                nc.scalar.activation(rstd[:], mv[:, 1:2],
                                     mybir.ActivationFunctionType.Sqrt,
                                     bias=epsln[:])
                nc.vector.reciprocal(rstd[:], rstd[:])
                nc.vector.tensor_scalar(xt[:], xt[:], mv[:, 0:1], rstd[:],
                                        A.subtract, A.mult)
                nc.vector.tensor_tensor(xt[:], xt[:], lng_row[:], A.mult)
                nc.vector.tensor_tensor(xt[:], xt[:], lnb_row[:], A.add)
                nc.sync.dma_start(out=y[r0:r1, :], in_=xt[:n, :])

    # run the bacc passes (wait splitting, register allocation) — the
    # run_bass_via_pjrt path does not call finalize() itself
    nc.finalize()
    return nc


def kernel(dst_feats, src_feats, edge_index, P_edge, deter_edge,
           W1, W2, W3, W4, Wv, Wout_w, Wout_b, res_w, res_b, ln_g, ln_b):
    dst_feats = np.ascontiguousarray(np.asarray(dst_feats, np.float32))
    src_feats = np.ascontiguousarray(np.asarray(src_feats, np.float32))
    W1 = np.asarray(W1, np.float32)
    W2 = np.asarray(W2, np.float32)
    W3 = np.asarray(W3, np.float32)
    W4 = np.asarray(W4, np.float32)
    Wv = np.asarray(Wv, np.float32)

    # tiny weight folds (O(D^2 * NH) on host)
    W14 = (W1 @ W4).astype(np.float32)
    W24 = (W2 @ W4).astype(np.float32)
    w34 = (W3[0] @ W4).astype(np.float32)
    wva = np.ascontiguousarray(
        np.concatenate([Wv, W24], axis=1).astype(np.float32))
    bias = (np.asarray(Wout_b, np.float32) + np.asarray(res_b, np.float32))

    C, ezb, eidx = _prep_edges(edge_index, P_edge, deter_edge, w34)

    nc = _build_program(C)

    in_maps = []
    for c in range(NCORES):
        s = slice(c * PER, (c + 1) * PER)
        in_maps.append({
            "srcf": np.ascontiguousarray(src_feats[s]),
            "dstf": np.ascontiguousarray(dst_feats[s]),
            "ezb": ezb[c],
            "eidx": eidx[c],
            "wva": wva,
            "w14": W14,
            "woutw": np.ascontiguousarray(np.asarray(Wout_w, np.float32)),
            "resw": np.ascontiguousarray(np.asarray(res_w, np.float32)),
            "biasv": bias,
            "lngv": np.asarray(ln_g, np.float32),
            "lnbv": np.asarray(ln_b, np.float32),
        })

    from concourse.bass_utils import run_bass_kernel_spmd
    res = run_bass_kernel_spmd(nc, in_maps, list(range(NCORES)))

    LAST_RUN["nc"] = nc
    LAST_RUN["in_maps"] = in_maps
    LAST_RUN["results"] = res

    out = np.concatenate([res.results[c]["y"] for c in range(NCORES)], axis=0)
    return out.astype(np.float32)


# revision 19
# speedup vs baseline: 9.6761x; 9.6761x over previous
"""Trainium2 Bass kernel for CrossSparseGAT message passing (8 NeuronCores).

Strategy (edge-parallel, dst-sorted):
  - Host: sort edges by dst, partition dst range across 8 cores (6250 dsts
    each), group each core's edges into 49 blocks of 128 dsts, pad each
    block's edge list to C chunks of 128 edges.
  - Device, per core:
      Phase A : project this core's src slice:  VA = src_feats @ [Wv | W2@W4]
      AllGather VA shards -> full VA table [50000, 136] on every core.
      Phase A2: a_dst = dst_feats_slice @ (W1@W4)  -> DRAM table [6250, 8]
      Phase C : per block: indirect-gather VA rows by src, CCE-add-gather
                a_dst rows by dst, compute per-edge softmax weights
                w = exp(leakyrelu(z)), scatter via one-hot matmul
                (S^T @ [w*V | w]) accumulated in PSUM over C chunks,
                normalize by the per-dst weight sum -> agg kept in SBUF.
      Phase D : out = agg @ Wout_w + dst_feats @ res_w + bias, LayerNorm.
  - Host: concatenate the 8 per-core output slices.

The segment softmax is computed without max-subtraction: exp(l - m) /
sum(exp(l - m)) == exp(l)/sum(exp(l)) and logits are O(10) here, so fp32
exp is safe (validated against an fp64 reference: rel err ~2e-7).
"""

import os

import numpy as np

N_DST = 50000
N_SRC = 50000
E = 500000
D = 128
NH = 8
HD = D // NH
NCORES = 8
PER = N_DST // NCORES          # 6250 dsts per core
NBLK = (PER + 127) // 128      # 49 blocks of 128 dsts
LAST_ROWS = PER - 128 * (NBLK - 1)  # rows in the last (partial) block
F_VA = D + NH                  # 136: [V | a_src] row size

# results of the last kernel() call, for the test harness
LAST_RUN = {}


def _prep_edges(edge_index, P_edge, deter_edge, w34):
    """Sort edges by dst, shard by dst range, pad per (core, block).

    Returns (C, ezb[8, NBLK, 128, C, 9] f32, eidx[8, NBLK, 128, C, 2] i32).
    ezb[..., 0:8] = P*w34 + deter (per-edge logit bias), ezb[..., 8] = dst
    offset within the block (999 marks padding -> one-hot row is all zero).
    eidx[..., 0] = global src index, eidx[..., 1] = dst index within core.
    """
    src = np.asarray(edge_index[0], dtype=np.int64)
    dst = np.asarray(edge_index[1], dtype=np.int64)
    order = np.argsort(dst, kind="stable")
    ssrc = src[order]
    sdst = dst[order]
    zb = (np.asarray(P_edge, np.float32)[order, None] * w34[None, :]
          + np.asarray(deter_edge, np.float32)[order, None]).astype(np.float32)

    core = sdst // PER
    local = sdst - core * PER
    blk = local // 128
    rel = local - blk * 128
    flat = core * NBLK + blk
    counts = np.bincount(flat, minlength=NCORES * NBLK)
    C = int(np.ceil(counts.max() / 128))
    C = max(C, 2)
    cap = C * 128

    start = np.zeros(NCORES * NBLK, np.int64)
    np.cumsum(counts[:-1], out=start[1:])
    slot = np.arange(E, dtype=np.int64) - start[flat]

    ezb = np.zeros((NCORES, NBLK, cap, 9), np.float32)
    ezb[:, :, :, 8] = 999.0
    eidx = np.zeros((NCORES, NBLK, cap, 2), np.int32)
    ezb[core, blk, slot, 0:8] = zb
    ezb[core, blk, slot, 8] = rel.astype(np.float32)
    eidx[core, blk, slot, 0] = ssrc.astype(np.int32)
    eidx[core, blk, slot, 1] = local.astype(np.int32)

    # device layouts: ezb [core, blk, p(=slot%128), chunk(=slot//128), f];
    # eidx [core, blk, p, f, chunk] so each index table is contiguous per
    # partition for the batched indirect DMA offset APs
    ezb = np.ascontiguousarray(
        ezb.reshape(NCORES, NBLK, C, 128, 9).transpose(0, 1, 3, 2, 4))
    eidx = np.ascontiguousarray(
        eidx.reshape(NCORES, NBLK, C, 128, 2).transpose(0, 1, 3, 4, 2))
    return C, ezb, eidx


def _build_program(C):
    import concourse.bass as bass
    import concourse.bacc as bacc
    import concourse.tile as tile
    from concourse import mybir
    from concourse.masks import make_identity

    f32 = mybir.dt.float32
    i32 = mybir.dt.int32
    A = mybir.AluOpType

    # timing-experiment knobs (debug only; default off -> full kernel)
    SKIP_GATHERS = os.environ.get("KV_SKIP_GATHERS") == "1"
    SKIP_AG = os.environ.get("KV_SKIP_AG") == "1"
    SKIP_EDGE = os.environ.get("KV_SKIP_EDGE") == "1"
    SKIP_DENSE = os.environ.get("KV_SKIP_DENSE") == "1"

    nc = bacc.Bacc(num_devices=NCORES)

    # --- I/O ---
    srcf = nc.dram_tensor("srcf", [PER, D], f32, kind="ExternalInput")
    dstf = nc.dram_tensor("dstf", [PER, D], f32, kind="ExternalInput")
    ezb = nc.dram_tensor("ezb", [NBLK, 128, C, 9], f32, kind="ExternalInput")
    eidx = nc.dram_tensor("eidx", [NBLK, 128, 2, C], i32, kind="ExternalInput")
    wva = nc.dram_tensor("wva", [D, F_VA], f32, kind="ExternalInput")
    w14 = nc.dram_tensor("w14", [D, NH], f32, kind="ExternalInput")
    woutw = nc.dram_tensor("woutw", [D, D], f32, kind="ExternalInput")
    resw = nc.dram_tensor("resw", [D, D], f32, kind="ExternalInput")
    biasv = nc.dram_tensor("biasv", [D], f32, kind="ExternalInput")
    lngv = nc.dram_tensor("lngv", [D], f32, kind="ExternalInput")
    lnbv = nc.dram_tensor("lnbv", [D], f32, kind="ExternalInput")
    y = nc.dram_tensor("y", [PER, D], f32, kind="ExternalOutput")

    def row_bcast(h):
        # DRAM [D] -> broadcast across 128 partitions
        ap = h[:]
        return bass.AP(tensor=ap.tensor, offset=ap.offset,
                       ap=[[0, 128]] + list(ap.ap))

    with tile.TileContext(nc) as tc:
        with (
            tc.tile_pool(name="consts", bufs=1) as consts,
            tc.tile_pool(name="aggp", bufs=1) as aggp,
            tc.tile_pool(name="densew", bufs=2) as densew,
            tc.tile_pool(name="edgew", bufs=3) as edgew,
            tc.tile_pool(name="psT", bufs=2, space="PSUM") as psT,
            tc.tile_pool(name="psMM", bufs=2, space="PSUM") as psMM,
            tc.tile_pool(name="psC", bufs=2, space="PSUM") as psC,
            tc.tile_pool(name="dram", bufs=1, space="DRAM") as dram,
        ):
            # --- constants ---
            ident = consts.tile([128, 128], f32)
            make_identity(nc, ident[:])
            iota_i = consts.tile([128, 128], i32)
            nc.gpsimd.iota(iota_i[:], pattern=[[1, 128]], base=0,
                           channel_multiplier=0)
            iota_f = consts.tile([128, 128], f32)
            nc.vector.tensor_copy(iota_f[:], iota_i[:])
            wva_sb = consts.tile([128, F_VA], f32)
            nc.sync.dma_start(out=wva_sb[:], in_=wva[:, :])
            w14_sb = consts.tile([128, NH], f32)
            nc.sync.dma_start(out=w14_sb[:], in_=w14[:, :])
            woutw_sb = consts.tile([128, D], f32)
            nc.sync.dma_start(out=woutw_sb[:], in_=woutw[:, :])
            resw_sb = consts.tile([128, D], f32)
            nc.sync.dma_start(out=resw_sb[:], in_=resw[:, :])
            bias_row = consts.tile([128, D], f32)
            nc.sync.dma_start(out=bias_row[:], in_=row_bcast(biasv))
            lng_row = consts.tile([128, D], f32)
            nc.sync.dma_start(out=lng_row[:], in_=row_bcast(lngv))
            lnb_row = consts.tile([128, D], f32)
            nc.sync.dma_start(out=lnb_row[:], in_=row_bcast(lnbv))
            eps12 = consts.tile([128, 1], f32)
            nc.vector.memset(eps12[:], 1e-12)
            epsln = consts.tile([128, 1], f32)
            nc.vector.memset(epsln[:], 1e-5)

            # SBUF-resident per-core aggregate [dst_in_block(part), blk*feat]
            aggbig = aggp.tile([128, NBLK * D], f32)

            # DRAM scratch
            va_sh = dram.tile([PER, F_VA], f32)
            va_full = dram.tile([N_SRC, F_VA], f32, addr_space="Shared")
            adst_loc = dram.tile([PER, NH], f32)

            # --- Phase A: VA shard = src_slice @ [Wv | W24] ---
            for t in range(NBLK):
                r0 = t * 128
                r1 = min(r0 + 128, PER)
                n = r1 - r0
                ft = densew.tile([128, D], f32, tag="ft")
                nc.sync.dma_start(out=ft[:n, :], in_=srcf[r0:r1, :])
                ftT_p = psT.tile([128, 128], f32, tag="tp")
                nc.tensor.transpose(ftT_p[:], ft[:], ident[:])
                ftT = densew.tile([128, 128], f32, tag="ftT")
                nc.vector.tensor_copy(ftT[:], ftT_p[:])
                va_p = psMM.tile([128, F_VA], f32, tag="mm")
                nc.tensor.matmul(va_p[:], lhsT=ftT[:], rhs=wva_sb[:],
                                 start=True, stop=True)
                va_sb = densew.tile([128, F_VA], f32, tag="vasb")
                nc.vector.tensor_copy(va_sb[:], va_p[:])
                nc.sync.dma_start(out=va_sh[r0:r1, :], in_=va_sb[:n, :])

            # --- AllGather the VA table ---
            if not SKIP_AG:
                nc.gpsimd.collective_compute(
                    "AllGather",
                    mybir.AluOpType.bypass,
                    replica_groups=[list(range(NCORES))],
                    ins=[va_sh[:].opt()],
                    outs=[va_full[:].opt()],
                )

            # --- Phase A2: a_dst table for this core's dst slice ---
            for t in range(NBLK):
                r0 = t * 128
                r1 = min(r0 + 128, PER)
                n = r1 - r0
                dt_ = densew.tile([128, D], f32, tag="ft")
                nc.sync.dma_start(out=dt_[:n, :], in_=dstf[r0:r1, :])
                dtT_p = psT.tile([128, 128], f32, tag="tp")
                nc.tensor.transpose(dtT_p[:], dt_[:], ident[:])
                dtT = densew.tile([128, 128], f32, tag="ftT")
                nc.vector.tensor_copy(dtT[:], dtT_p[:])
                ad_p = psMM.tile([128, NH], f32, tag="mm")
                nc.tensor.matmul(ad_p[:], lhsT=dtT[:], rhs=w14_sb[:],
                                 start=True, stop=True)
                ad_sb = densew.tile([128, NH], f32, tag="adsb")
                nc.vector.tensor_copy(ad_sb[:], ad_p[:])
                nc.sync.dma_start(out=adst_loc[r0:r1, :], in_=ad_sb[:n, :])

            # --- Phase C: edge processing, one block of 128 dsts at a time ---
            if SKIP_EDGE:
                nc.vector.memset(aggbig[:], 0.0)
            for b in range(0 if not SKIP_EDGE else NBLK, NBLK):
                ez = edgew.tile([128, C, 9], f32, tag="ez")
                nc.sync.dma_start(out=ez[:], in_=ezb[b])
                ei = edgew.tile([128, 2, C], i32, tag="ei")
                nc.sync.dma_start(out=ei[:], in_=eidx[b])

                # per-chunk indirect gathers ([128, 1] offsets only — HW
                # does not honor multi-column offset APs)
                vab = edgew.tile([128, C, F_VA], f32, tag="vab")
                if SKIP_GATHERS:
                    nc.vector.memset(vab[:], 1.0)
                if not SKIP_GATHERS:
                    for k in range(C):
                        nc.gpsimd.indirect_dma_start(
                            out=vab[:, k, :],
                            out_offset=None,
                            in_=va_full[:],
                            in_offset=bass.IndirectOffsetOnAxis(
                                ap=ei[:, 0, k:k + 1], axis=0),
                        )
                    for k in range(C):
                        nc.gpsimd.indirect_dma_start(
                            out=vab[:, k, D:F_VA],
                            out_offset=None,
                            in_=adst_loc[:],
                            in_offset=bass.IndirectOffsetOnAxis(
                                ap=ei[:, 1, k:k + 1], axis=0),
                            compute_op=A.add,
                        )

                # z = (P*w34 + deter) + (a_src + a_dst);  l = max(z, 0.2 z)
                zt = edgew.tile([128, C, NH], f32, tag="zt")
                nc.vector.tensor_tensor(zt[:], ez[:, :, 0:8],
                                        vab[:, :, D:F_VA], A.add)
                lt = edgew.tile([128, C, NH], f32, tag="lt")
                nc.vector.scalar_tensor_tensor(lt[:], zt[:], 0.2, zt[:],
                                               A.mult, A.max)
                pay = edgew.tile([128, C, F_VA], f32, tag="pay")
                nc.scalar.activation(pay[:, :, D:F_VA], lt[:],
                                     mybir.ActivationFunctionType.Exp)
                # msgs = w (per head) * V
                nc.vector.tensor_tensor(
                    pay[:, :, 0:D].rearrange("p c (h j) -> p c h j", h=NH),
                    vab[:, :, 0:D].rearrange("p c (h j) -> p c h j", h=NH),
                    pay[:, :, D:F_VA].unsqueeze(3).to_broadcast(
                        [128, C, NH, HD]),
                    A.mult)
                # one-hot S[e, d] = (dst_rel[e] == d)
                St = edgew.tile([128, C, 128], f32, tag="St")
                nc.vector.tensor_tensor(
                    St[:],
                    ez[:, :, 8:9].to_broadcast([128, C, 128]),
                    iota_f[:].unsqueeze(1).to_broadcast([128, C, 128]),
                    A.is_equal)

                ps = psC.tile([128, F_VA], f32, tag="ps")
                for k in range(C):
                    nc.tensor.matmul(ps[:], lhsT=St[:, k, :], rhs=pay[:, k, :],
                                     start=(k == 0), stop=(k == C - 1))

                # normalize: agg = U / (ssum + 1e-12)
                rec = edgew.tile([128, NH], f32, tag="rec")
                nc.scalar.activation(rec[:], ps[:, D:F_VA],
                                     mybir.ActivationFunctionType.Identity,
                                     bias=eps12[:])
                nc.vector.reciprocal(rec[:], rec[:])
                nc.vector.tensor_tensor(
                    aggbig[:, b * D:(b + 1) * D].rearrange(
                        "p (h j) -> p h j", h=NH),
                    ps[:, 0:D].rearrange("p (h j) -> p h j", h=NH),
                    rec[:].unsqueeze(2).to_broadcast([128, NH, HD]),
                    A.mult)

            # --- Phase D: out = agg @ Wout_w + dstf @ res_w + bias; LayerNorm
            if SKIP_DENSE:
                nc.sync.dma_start(out=y[:, :], in_=dstf[:, :])
            for t in range(NBLK if not SKIP_DENSE else 0):
                r0 = t * 128
                r1 = min(r0 + 128, PER)
                n = r1 - r0
                agT_p = psT.tile([128, 128], f32, tag="tp")
                nc.tensor.transpose(agT_p[:], aggbig[:, t * D:(t + 1) * D],
                                    ident[:])
                agT = densew.tile([128, 128], f32, tag="ftT")
                nc.vector.tensor_copy(agT[:], agT_p[:])
                dt_ = densew.tile([128, D], f32, tag="ft")
                nc.sync.dma_start(out=dt_[:n, :], in_=dstf[r0:r1, :])
                dtT_p = psT.tile([128, 128], f32, tag="tp")
                nc.tensor.transpose(dtT_p[:], dt_[:], ident[:])
                dtT = densew.tile([128, 128], f32, tag="ftT2")
                nc.vector.tensor_copy(dtT[:], dtT_p[:])
                op = psMM.tile([128, D], f32, tag="mm")
                nc.tensor.matmul(op[:], lhsT=agT[:], rhs=woutw_sb[:],
                                 start=True, stop=False)
                nc.tensor.matmul(op[:], lhsT=dtT[:], rhs=resw_sb[:],
                                 start=False, stop=True)
                xt = densew.tile([128, D], f32, tag="xt")
                nc.vector.tensor_tensor(xt[:], op[:], bias_row[:], A.add)
                stats = densew.tile([128, nc.vector.BN_STATS_DIM], f32,
                                    tag="stats")
                nc.vector.bn_stats(stats[:], xt[:])
                mv = densew.tile([128, nc.vector.BN_AGGR_DIM], f32, tag="mv")
                nc.vector.bn_aggr(mv[:], stats[:])
                rstd = densew.tile([128, 1], f32, tag="rstd")
                nc.scalar.activation(rstd[:], mv[:, 1:2],
                                     mybir.ActivationFunctionType.Sqrt,
                                     bias=epsln[:])
                nc.vector.reciprocal(rstd[:], rstd[:])
                nc.vector.tensor_scalar(xt[:], xt[:], mv[:, 0:1], rstd[:],
                                        A.subtract, A.mult)
                nc.vector.tensor_tensor(xt[:], xt[:], lng_row[:], A.mult)
                nc.vector.tensor_tensor(xt[:], xt[:], lnb_row[:], A.add)
                nc.sync.dma_start(out=y[r0:r1, :], in_=xt[:n, :])

    # run the bacc passes (wait splitting, register allocation) — the
    # run_bass_via_pjrt path does not call finalize() itself
    nc.finalize()
    return nc


def kernel(dst_feats, src_feats, edge_index, P_edge, deter_edge,
           W1, W2, W3, W4, Wv, Wout_w, Wout_b, res_w, res_b, ln_g, ln_b):
    dst_feats = np.ascontiguousarray(np.asarray(dst_feats, np.float32))
    src_feats = np.ascontiguousarray(np.asarray(src_feats, np.float32))
    W1 = np.asarray(W1, np.float32)
    W2 = np.asarray(W2, np.float32)
    W3 = np.asarray(W3, np.float32)
    W4 = np.asarray(W4, np.float32)
    Wv = np.asarray(Wv, np.float32)

    # tiny weight folds (O(D^2 * NH) on host)
    W14 = (W1 @ W4).astype(np.float32)
    W24 = (W2 @ W4).astype(np.float32)
    w34 = (W3[0] @ W4).astype(np.float32)
    wva = np.ascontiguousarray(
        np.concatenate([Wv, W24], axis=1).astype(np.float32))
    bias = (np.asarray(Wout_b, np.float32) + np.asarray(res_b, np.float32))

    C, ezb, eidx = _prep_edges(edge_index, P_edge, deter_edge, w34)

    nc = _build_program(C)

    in_maps = []
    for c in range(NCORES):
        s = slice(c * PER, (c + 1) * PER)
        in_maps.append({
            "srcf": np.ascontiguousarray(src_feats[s]),
            "dstf": np.ascontiguousarray(dst_feats[s]),
            "ezb": ezb[c],
            "eidx": eidx[c],
            "wva": wva,
            "w14": W14,
            "woutw": np.ascontiguousarray(np.asarray(Wout_w, np.float32)),
            "resw": np.ascontiguousarray(np.asarray(res_w, np.float32)),
            "biasv": bias,
            "lngv": np.asarray(ln_g, np.float32),
            "lnbv": np.asarray(ln_b, np.float32),
        })

    from concourse.bass_utils import run_bass_kernel_spmd
    res = run_bass_kernel_spmd(nc, in_maps, list(range(NCORES)))

    LAST_RUN["nc"] = nc
    LAST_RUN["in_maps"] = in_maps
    LAST_RUN["results"] = res

    out = np.concatenate([res.results[c]["y"] for c in range(NCORES)], axis=0)
    return out.astype(np.float32)


# revision 23
# speedup vs baseline: 11.0436x; 1.1413x over previous
"""Trainium2 Bass kernel for CrossSparseGAT message passing (8 NeuronCores).

Strategy (edge-parallel, dst-sorted):
  - Host: sort edges by dst, partition dst range across 8 cores (6250 dsts
    each), group each core's edges into 49 blocks of 128 dsts, pad each
    block's edge list to C chunks of 128 edges.
  - Device, per core:
      Phase A : project this core's src slice:  VA = src_feats @ [Wv | W2@W4]
      AllGather VA shards -> full VA table [50000, 136] on every core.
      Phase A2: a_dst = dst_feats_slice @ (W1@W4)  -> DRAM table [6250, 8]
      Phase C : per block: indirect-gather VA rows by src, CCE-add-gather
                a_dst rows by dst, compute per-edge softmax weights
                w = exp(leakyrelu(z)), scatter via one-hot matmul
                (S^T @ [w*V | w]) accumulated in PSUM over C chunks,
                normalize by the per-dst weight sum -> agg kept in SBUF.
      Phase D : out = agg @ Wout_w + dst_feats @ res_w + bias, LayerNorm.
  - Host: concatenate the 8 per-core output slices.

The segment softmax is computed without max-subtraction: exp(l - m) /
sum(exp(l - m)) == exp(l)/sum(exp(l)) and logits are O(10) here, so fp32
exp is safe (validated against an fp64 reference: rel err ~2e-7).
"""

import os

import numpy as np

N_DST = 50000
N_SRC = 50000
E = 500000
D = 128
NH = 8
HD = D // NH
NCORES = 8
PER = N_DST // NCORES          # 6250 dsts per core
NBLK = (PER + 127) // 128      # 49 blocks of 128 dsts
LAST_ROWS = PER - 128 * (NBLK - 1)  # rows in the last (partial) block
F_VA = D + NH                  # 136: [V | a_src] row size

# results of the last kernel() call, for the test harness
LAST_RUN = {}


def _prep_edges(edge_index, P_edge, deter_edge, w34):
    """Sort edges by dst, shard by dst range, pad per (core, block).

    Returns (C, ezb[8, NBLK, 128, C, 9] f32, eidx[8, NBLK, 128, C, 2] i32).
    ezb[..., 0:8] = P*w34 + deter (per-edge logit bias), ezb[..., 8] = dst
    offset within the block (999 marks padding -> one-hot row is all zero).
    eidx[..., 0] = global src index, eidx[..., 1] = dst index within core.
    """
    src = np.asarray(edge_index[0], dtype=np.int64)
    dst = np.asarray(edge_index[1], dtype=np.int64)
    order = np.argsort(dst, kind="stable")
    ssrc = src[order]
    sdst = dst[order]
    zb = (np.asarray(P_edge, np.float32)[order, None] * w34[None, :]
          + np.asarray(deter_edge, np.float32)[order, None]).astype(np.float32)

    core = sdst // PER
    local = sdst - core * PER
    blk = local // 128
    rel = local - blk * 128
    flat = core * NBLK + blk
    counts = np.bincount(flat, minlength=NCORES * NBLK)
    C = int(np.ceil(counts.max() / 128))
    C = max(C, 2)
    cap = C * 128

    start = np.zeros(NCORES * NBLK, np.int64)
    np.cumsum(counts[:-1], out=start[1:])
    slot = np.arange(E, dtype=np.int64) - start[flat]

    ezb = np.zeros((NCORES, NBLK, cap, 9), np.float32)
    ezb[:, :, :, 8] = 999.0
    eidx = np.zeros((NCORES, NBLK, cap, 2), np.int32)
    ezb[core, blk, slot, 0:8] = zb
    ezb[core, blk, slot, 8] = rel.astype(np.float32)
    eidx[core, blk, slot, 0] = ssrc.astype(np.int32)
    eidx[core, blk, slot, 1] = local.astype(np.int32)

    # device layouts: ezb [core, blk, p(=slot%128), chunk(=slot//128), f];
    # eidx [core, blk, p, f, chunk] so each index table is contiguous per
    # partition for the batched indirect DMA offset APs
    ezb = np.ascontiguousarray(
        ezb.reshape(NCORES, NBLK, C, 128, 9).transpose(0, 1, 3, 2, 4))
    eidx = np.ascontiguousarray(
        eidx.reshape(NCORES, NBLK, C, 128, 2).transpose(0, 1, 3, 4, 2))
    return C, ezb, eidx


def _build_program(C):
    import concourse.bass as bass
    import concourse.bacc as bacc
    import concourse.tile as tile
    from concourse import mybir
    from concourse.masks import make_identity

    f32 = mybir.dt.float32
    i32 = mybir.dt.int32
    A = mybir.AluOpType

    # timing-experiment knobs (debug only; default off -> full kernel)
    SKIP_GATHERS = os.environ.get("KV_SKIP_GATHERS") == "1"
    SKIP_AG = os.environ.get("KV_SKIP_AG") == "1"
    SKIP_EDGE = os.environ.get("KV_SKIP_EDGE") == "1"
    SKIP_DENSE = os.environ.get("KV_SKIP_DENSE") == "1"

    nc = bacc.Bacc(num_devices=NCORES)

    # --- I/O ---
    srcf = nc.dram_tensor("srcf", [PER, D], f32, kind="ExternalInput")
    dstf = nc.dram_tensor("dstf", [PER, D], f32, kind="ExternalInput")
    ezb = nc.dram_tensor("ezb", [NBLK, 128, C, 9], f32, kind="ExternalInput")
    eidx = nc.dram_tensor("eidx", [NBLK, 128, 2, C], i32, kind="ExternalInput")
    wva = nc.dram_tensor("wva", [D, F_VA], f32, kind="ExternalInput")
    w14 = nc.dram_tensor("w14", [D, NH], f32, kind="ExternalInput")
    woutw = nc.dram_tensor("woutw", [D, D], f32, kind="ExternalInput")
    resw = nc.dram_tensor("resw", [D, D], f32, kind="ExternalInput")
    biasv = nc.dram_tensor("biasv", [D], f32, kind="ExternalInput")
    lngv = nc.dram_tensor("lngv", [D], f32, kind="ExternalInput")
    lnbv = nc.dram_tensor("lnbv", [D], f32, kind="ExternalInput")
    y = nc.dram_tensor("y", [PER, D], f32, kind="ExternalOutput")

    def row_bcast(h):
        # DRAM [D] -> broadcast across 128 partitions
        ap = h[:]
        return bass.AP(tensor=ap.tensor, offset=ap.offset,
                       ap=[[0, 128]] + list(ap.ap))

    with tile.TileContext(nc) as tc:
        with (
            tc.tile_pool(name="consts", bufs=1) as consts,
            tc.tile_pool(name="aggp", bufs=1) as aggp,
            tc.tile_pool(name="densew", bufs=2) as densew,
            tc.tile_pool(name="edgew", bufs=3) as edgew,
            tc.tile_pool(name="psT", bufs=2, space="PSUM") as psT,
            tc.tile_pool(name="psMM", bufs=2, space="PSUM") as psMM,
            tc.tile_pool(name="psC", bufs=2, space="PSUM") as psC,
            tc.tile_pool(name="dram", bufs=1, space="DRAM") as dram,
        ):
            # --- constants ---
            ident = consts.tile([128, 128], f32)
            make_identity(nc, ident[:])
            iota_i = consts.tile([128, 128], i32)
            nc.gpsimd.iota(iota_i[:], pattern=[[1, 128]], base=0,
                           channel_multiplier=0)
            iota_f = consts.tile([128, 128], f32)
            nc.vector.tensor_copy(iota_f[:], iota_i[:])
            wva_sb = consts.tile([128, F_VA], f32)
            nc.sync.dma_start(out=wva_sb[:], in_=wva[:, :])
            w14_sb = consts.tile([128, NH], f32)
            nc.sync.dma_start(out=w14_sb[:], in_=w14[:, :])
            woutw_sb = consts.tile([128, D], f32)
            nc.sync.dma_start(out=woutw_sb[:], in_=woutw[:, :])
            resw_sb = consts.tile([128, D], f32)
            nc.sync.dma_start(out=resw_sb[:], in_=resw[:, :])
            bias_row = consts.tile([128, D], f32)
            nc.sync.dma_start(out=bias_row[:], in_=row_bcast(biasv))
            lng_row = consts.tile([128, D], f32)
            nc.sync.dma_start(out=lng_row[:], in_=row_bcast(lngv))
            lnb_row = consts.tile([128, D], f32)
            nc.sync.dma_start(out=lnb_row[:], in_=row_bcast(lnbv))
            eps12 = consts.tile([128, 1], f32)
            nc.vector.memset(eps12[:], 1e-12)
            epsln = consts.tile([128, 1], f32)
            nc.vector.memset(epsln[:], 1e-5)

            # SBUF-resident per-core aggregate [dst_in_block(part), blk*feat]
            aggbig = aggp.tile([128, NBLK * D], f32)
            # SBUF-resident a_dst table [dst_in_block(part), blk*NH]
            adbig = aggp.tile([128, NBLK * NH], f32)

            # DRAM scratch
            va_sh = dram.tile([PER, F_VA], f32)
            va_full = dram.tile([N_SRC, F_VA], f32, addr_space="Shared")


            # --- Phase A: VA shard = src_slice @ [Wv | W24] ---
            for t in range(NBLK):
                r0 = t * 128
                r1 = min(r0 + 128, PER)
                n = r1 - r0
                ft = densew.tile([128, D], f32, tag="ft")
                nc.sync.dma_start(out=ft[:n, :], in_=srcf[r0:r1, :])
                ftT_p = psT.tile([128, 128], f32, tag="tp")
                nc.tensor.transpose(ftT_p[:], ft[:], ident[:])
                ftT = densew.tile([128, 128], f32, tag="ftT")
                nc.vector.tensor_copy(ftT[:], ftT_p[:])
                va_p = psMM.tile([128, F_VA], f32, tag="mm")
                nc.tensor.matmul(va_p[:], lhsT=ftT[:], rhs=wva_sb[:],
                                 start=True, stop=True)
                va_sb = densew.tile([128, F_VA], f32, tag="vasb")
                nc.vector.tensor_copy(va_sb[:], va_p[:])
                nc.sync.dma_start(out=va_sh[r0:r1, :], in_=va_sb[:n, :])

            # --- AllGather the VA table ---
            if not SKIP_AG:
                nc.gpsimd.collective_compute(
                    "AllGather",
                    mybir.AluOpType.bypass,
                    replica_groups=[list(range(NCORES))],
                    ins=[va_sh[:].opt()],
                    outs=[va_full[:].opt()],
                )

            # --- Phase A2: a_dst table for this core's dst slice ---
            for t in range(NBLK):
                r0 = t * 128
                r1 = min(r0 + 128, PER)
                n = r1 - r0
                dt_ = densew.tile([128, D], f32, tag="ft")
                nc.sync.dma_start(out=dt_[:n, :], in_=dstf[r0:r1, :])
                dtT_p = psT.tile([128, 128], f32, tag="tp")
                nc.tensor.transpose(dtT_p[:], dt_[:], ident[:])
                dtT = densew.tile([128, 128], f32, tag="ftT")
                nc.vector.tensor_copy(dtT[:], dtT_p[:])
                ad_p = psMM.tile([128, NH], f32, tag="mm")
                nc.tensor.matmul(ad_p[:], lhsT=dtT[:], rhs=w14_sb[:],
                                 start=True, stop=True)
                nc.vector.tensor_copy(adbig[:, t * NH:(t + 1) * NH], ad_p[:])

            # --- Phase C: edge processing, one block of 128 dsts at a time ---
            if SKIP_EDGE:
                nc.vector.memset(aggbig[:], 0.0)
            for b in range(0 if not SKIP_EDGE else NBLK, NBLK):
                ez = edgew.tile([128, C, 9], f32, tag="ez")
                nc.sync.dma_start(out=ez[:], in_=ezb[b])
                ei = edgew.tile([128, 2, C], i32, tag="ei")
                nc.sync.dma_start(out=ei[:], in_=eidx[b])

                # per-chunk indirect gathers ([128, 1] offsets only — HW
                # does not honor multi-column offset APs)
                vab = edgew.tile([128, C, F_VA], f32, tag="vab")
                if SKIP_GATHERS:
                    nc.vector.memset(vab[:], 1.0)
                if not SKIP_GATHERS:
                    for k in range(C):
                        nc.gpsimd.indirect_dma_start(
                            out=vab[:, k, :],
                            out_offset=None,
                            in_=va_full[:],
                            in_offset=bass.IndirectOffsetOnAxis(
                                ap=ei[:, 0, k:k + 1], axis=0),
                        )
                # one-hot S[e, d] = (dst_rel[e] == d)
                St = edgew.tile([128, C, 128], f32, tag="St")
                nc.vector.tensor_tensor(
                    St[:],
                    ez[:, :, 8:9].to_broadcast([128, C, 128]),
                    iota_f[:].unsqueeze(1).to_broadcast([128, C, 128]),
                    A.is_equal)

                # broadcast a_dst to edges: adE[:, k, :] = S_ed @ adbig_blk,
                # computed as (S_ed^T).T @ adbig_blk with a PE transpose —
                # replaces 539 per-chunk indirect add-gathers
                adE = edgew.tile([128, C, NH], f32, tag="adE")
                for k in range(C):
                    StT_p = psT.tile([128, 128], f32, tag="tp")
                    nc.tensor.transpose(StT_p[:], St[:, k, :], ident[:])
                    StT = edgew.tile([128, 128], f32, tag="StT")
                    nc.vector.tensor_copy(StT[:], StT_p[:])
                    ad_p = psMM.tile([128, NH], f32, tag="mm")
                    nc.tensor.matmul(
                        ad_p[:], lhsT=StT[:],
                        rhs=adbig[:, b * NH:(b + 1) * NH],
                        start=True, stop=True)
                    nc.vector.tensor_copy(adE[:, k, :], ad_p[:])

                # z = (P*w34 + deter) + a_src + a_dst;  l = max(z, 0.2 z)
                zt = edgew.tile([128, C, NH], f32, tag="zt")
                nc.vector.tensor_tensor(zt[:], ez[:, :, 0:8],
                                        vab[:, :, D:F_VA], A.add)
                nc.vector.tensor_tensor(zt[:], zt[:], adE[:], A.add)
                lt = edgew.tile([128, C, NH], f32, tag="lt")
                nc.vector.scalar_tensor_tensor(lt[:], zt[:], 0.2, zt[:],
                                               A.mult, A.max)
                pay = edgew.tile([128, C, F_VA], f32, tag="pay")
                nc.scalar.activation(pay[:, :, D:F_VA], lt[:],
                                     mybir.ActivationFunctionType.Exp)
                # msgs = w (per head) * V
                nc.vector.tensor_tensor(
                    pay[:, :, 0:D].rearrange("p c (h j) -> p c h j", h=NH),
                    vab[:, :, 0:D].rearrange("p c (h j) -> p c h j", h=NH),
                    pay[:, :, D:F_VA].unsqueeze(3).to_broadcast(
                        [128, C, NH, HD]),
                    A.mult)

                ps = psC.tile([128, F_VA], f32, tag="ps")
                for k in range(C):
                    nc.tensor.matmul(ps[:], lhsT=St[:, k, :], rhs=pay[:, k, :],
                                     start=(k == 0), stop=(k == C - 1))

                # normalize: agg = U / (ssum + 1e-12)
                rec = edgew.tile([128, NH], f32, tag="rec")
                nc.scalar.activation(rec[:], ps[:, D:F_VA],
                                     mybir.ActivationFunctionType.Identity,
                                     bias=eps12[:])
                nc.vector.reciprocal(rec[:], rec[:])
                nc.vector.tensor_tensor(
                    aggbig[:, b * D:(b + 1) * D].rearrange(
                        "p (h j) -> p h j", h=NH),
                    ps[:, 0:D].rearrange("p (h j) -> p h j", h=NH),
                    rec[:].unsqueeze(2).to_broadcast([128, NH, HD]),
                    A.mult)

            # --- Phase D: out = agg @ Wout_w + dstf @ res_w + bias; LayerNorm
            if SKIP_DENSE:
                nc.sync.dma_start(out=y[:, :], in_=dstf[:, :])
            for t in range(NBLK if not SKIP_DENSE else 0):
                r0 = t * 128
                r1 = min(r0 + 128, PER)
                n = r1 - r0
                agT_p = psT.tile([128, 128], f32, tag="tp")
                nc.tensor.transpose(agT_p[:], aggbig[:, t * D:(t + 1) * D],
                                    ident[:])
                agT = densew.tile([128, 128], f32, tag="ftT")
                nc.vector.tensor_copy(agT[:], agT_p[:])
                dt_ = densew.tile([128, D], f32, tag="ft")
                nc.sync.dma_start(out=dt_[:n, :], in_=dstf[r0:r1, :])
                dtT_p = psT.tile([128, 128], f32, tag="tp")
                nc.tensor.transpose(dtT_p[:], dt_[:], ident[:])
                dtT = densew.tile([128, 128], f32, tag="ftT2")
                nc.vector.tensor_copy(dtT[:], dtT_p[:])
                op = psMM.tile([128, D], f32, tag="mm")
                nc.tensor.matmul(op[:], lhsT=agT[:], rhs=woutw_sb[:],
                                 start=True, stop=False)
                nc.tensor.matmul(op[:], lhsT=dtT[:], rhs=resw_sb[:],
                                 start=False, stop=True)
                xt = densew.tile([128, D], f32, tag="xt")
                nc.vector.tensor_tensor(xt[:], op[:], bias_row[:], A.add)
                stats = densew.tile([128, nc.vector.BN_STATS_DIM], f32,
                                    tag="stats")
                nc.vector.bn_stats(stats[:], xt[:])
                mv = densew.tile([128, nc.vector.BN_AGGR_DIM], f32, tag="mv")
                nc.vector.bn_aggr(mv[:], stats[:])
                rstd = densew.tile([128, 1], f32, tag="rstd")
                nc.scalar.activation(rstd[:], mv[:, 1:2],
                                     mybir.ActivationFunctionType.Sqrt,
                                     bias=epsln[:])
                nc.vector.reciprocal(rstd[:], rstd[:])
                nc.vector.tensor_scalar(xt[:], xt[:], mv[:, 0:1], rstd[:],
                                        A.subtract, A.mult)
                nc.vector.tensor_tensor(xt[:], xt[:], lng_row[:], A.mult)
                nc.vector.tensor_tensor(xt[:], xt[:], lnb_row[:], A.add)
                nc.sync.dma_start(out=y[r0:r1, :], in_=xt[:n, :])

    # run the bacc passes (wait splitting, register allocation) — the
    # run_bass_via_pjrt path does not call finalize() itself
    nc.finalize()
    return nc


def kernel(dst_feats, src_feats, edge_index, P_edge, deter_edge,
           W1, W2, W3, W4, Wv, Wout_w, Wout_b, res_w, res_b, ln_g, ln_b):
    dst_feats = np.ascontiguousarray(np.asarray(dst_feats, np.float32))
    src_feats = np.ascontiguousarray(np.asarray(src_feats, np.float32))
    W1 = np.asarray(W1, np.float32)
    W2 = np.asarray(W2, np.float32)
    W3 = np.asarray(W3, np.float32)
    W4 = np.asarray(W4, np.float32)
    Wv = np.asarray(Wv, np.float32)

    # tiny weight folds (O(D^2 * NH) on host)
    W14 = (W1 @ W4).astype(np.float32)
    W24 = (W2 @ W4).astype(np.float32)
    w34 = (W3[0] @ W4).astype(np.float32)
    wva = np.ascontiguousarray(
        np.concatenate([Wv, W24], axis=1).astype(np.float32))
    bias = (np.asarray(Wout_b, np.float32) + np.asarray(res_b, np.float32))

    C, ezb, eidx = _prep_edges(edge_index, P_edge, deter_edge, w34)

    nc = _build_program(C)

    in_maps = []
    for c in range(NCORES):
        s = slice(c * PER, (c + 1) * PER)
        in_maps.append({
            "srcf": np.ascontiguousarray(src_feats[s]),
            "dstf": np.ascontiguousarray(dst_feats[s]),
            "ezb": ezb[c],
            "eidx": eidx[c],
            "wva": wva,
            "w14": W14,
            "woutw": np.ascontiguousarray(np.asarray(Wout_w, np.float32)),
            "resw": np.ascontiguousarray(np.asarray(res_w, np.float32)),
            "biasv": bias,
            "lngv": np.asarray(ln_g, np.float32),
            "lnbv": np.asarray(ln_b, np.float32),
        })

    from concourse.bass_utils import run_bass_kernel_spmd
    res = run_bass_kernel_spmd(nc, in_maps, list(range(NCORES)))

    LAST_RUN["nc"] = nc
    LAST_RUN["in_maps"] = in_maps
    LAST_RUN["results"] = res

    out = np.concatenate([res.results[c]["y"] for c in range(NCORES)], axis=0)
    return out.astype(np.float32)
